# revision 3
# baseline (speedup 1.0000x reference)
"""v6: v2 + partial-width diagonal score blocks. Multi-head causal attention (B=2, T=2048, C=4096, H=32) on 8 Trainium2
NeuronCores, tensor-parallel over heads (Megatron-style). v2.

Per core m (4 heads each):
  phase 1: q/k/v projections from full x (weights column-sharded, host
           pre-transposed to lhsT layout, loaded in k-chunks so the first
           matmul starts ~10us in). RoPE at PSUM eviction (all rotary freqs
           == 1.0 here, so cos/sin are per-position scalars; head_dim is
           host-permuted to [evens, odds] so rotation pairs sit in partition
           halves; the half-swap runs through SBUF->SBUF DMA).
  phase 2: attention per (head, batch) with scores TRANSPOSED [k, q]:
           u = exp(scale * sT) (no max subtraction needed at these scales),
           causal-masked; o.T = v.T @ probs.T accumulates in PSUM; softmax
           denominator via an all-ones stationary matmul. The kt loop is
           software-pipelined 2 deep so the PE never waits on the exp.
  phase 3: two AllToAlls (one per batch element) redistribute o.T so each
           core owns ALL heads for a 256-row slice of each batch; the b=0
           collective and its output projection overlap with b=1 attention.
           y rows [0:256) = batch 0 slice, [256:512) = batch 1 slice.
Host gathers the 16 row-slices. Host does layout prep (transpose/cast) and
the final scatter only.
"""

import os
import sys

import numpy as np

for _p in ("/opt/trn_rl_repo", "/root/.axon_site/_ro/trn_rl_repo"):
    if os.path.isdir(_p) and _p not in sys.path:
        sys.path.insert(0, _p)

import ml_dtypes

import concourse.bacc as bacc
import concourse.bass as bass
import concourse.mybir as mybir
import concourse.tile as tile
from concourse.bass_utils import run_bass_kernel_spmd

BF16 = ml_dtypes.bfloat16
P = 128
NCORES = 8
DT = mybir.dt.bfloat16
F32 = mybir.dt.float32
ActFn = mybir.ActivationFunctionType

FULL = dict(B=2, T=2048, C=4096, H=32, W=512, QT=512)


def _dims(cfg):
    B, T, C, H = cfg["B"], cfg["T"], cfg["C"], cfg["H"]
    W, QT = cfg["W"], cfg["QT"]
    HD = C // H
    assert HD == P
    HL = H // NCORES
    R = B * T
    RS = R // NCORES
    KO = C // P
    return B, T, C, H, HD, HL, R, RS, KO, W, QT


def build_nc(cfg=FULL, repeat=1, use_a2a=True):
    B, T, C, H, HD, HL, R, RS, KO, W, QT = _dims(cfg)
    NW = R // W
    NKT = T // P
    NCB = C // QT
    SCALE = float(HD) ** -0.5
    MOFF = QT - P
    KO2 = KO // 2
    KCH = KO // 4           # weight-load chunk
    RS2 = RS // B           # rows per (core, batch)

    nc = bacc.Bacc(None, num_devices=NCORES)

    xT = nc.dram_tensor("xT", [P, NW, KO, W], DT, kind="ExternalInput")
    wqT = nc.dram_tensor("wqT", [P, KO, HL * HD], DT, kind="ExternalInput")
    wkT = nc.dram_tensor("wkT", [P, KO, HL * HD], DT, kind="ExternalInput")
    wvT = nc.dram_tensor("wvT", [P, KO, HL * HD], DT, kind="ExternalInput")
    woT = nc.dram_tensor("woT", [P, NCB, KO, QT], DT, kind="ExternalInput")
    cosR = nc.dram_tensor("cosR", [P, R], DT, kind="ExternalInput")
    sinS = nc.dram_tensor("sinS", [P, R], DT, kind="ExternalInput")
    maskb = nc.dram_tensor("maskb", [P, MOFF + QT], DT, kind="ExternalInput")
    y = nc.dram_tensor("y", [RS, C], F32, kind="ExternalOutput")

    qT_d = nc.dram_tensor("qT_d", [P, HL, R], DT)
    kT_d = nc.dram_tensor("kT_d", [P, HL, R], DT)
    v_d = nc.dram_tensor("v_d", [P, R // P, HL * HD], DT)
    a2a_i = [nc.dram_tensor(f"a2a_i{b}", [NCORES, HL * HD, RS2], DT)
             for b in range(B)]
    a2a_o = [nc.dram_tensor(f"a2a_o{b}", [NCORES, HL * HD, RS2], DT)
             for b in range(B)]

    with tile.TileContext(nc) as tc:
      for _rep in range(repeat):
        # ---------------- phase 1: q/k/v projections + rope ----------------
        with (
            tc.tile_pool(name="wp", bufs=1) as wp,
            tc.tile_pool(name="tab1", bufs=1) as tab1,
            tc.tile_pool(name="xp", bufs=2) as xp,
            tc.tile_pool(name="ev1", bufs=3) as ev1,
            tc.tile_pool(name="ps1", bufs=2, space="PSUM") as ps1,
        ):
            wq_sb = wp.tile([P, KO, HL * HD], DT, tag="wq")
            wk_sb = wp.tile([P, KO, HL * HD], DT, tag="wk")
            wv_sb = wp.tile([P, KO, HL * HD], DT, tag="wv")
            cos_sb = tab1.tile([P, R], DT, tag="cos")
            sin_sb = tab1.tile([P, R], DT, tag="sin")
            for c in range(KO // KCH):
                ksl = slice(c * KCH, (c + 1) * KCH)
                nc.gpsimd.dma_start(wq_sb[:, ksl], wqT[:, ksl])

            def qk_proj(wsb, dst, h, xa, xb, rsl):
                pt = ps1.tile([P, W], F32, tag="pqk")
                for k in range(KO):
                    xk_ = xa[:, k] if k < KO2 else xb[:, k - KO2]
                    nc.tensor.matmul(
                        pt[:], wsb[:, k, h * HD:(h + 1) * HD], xk_,
                        start=(k == 0), stop=(k == KO - 1),
                    )
                # rope: rot = raw*cos + swap(raw)*sinS (sign-split sin); the
                # partition half-swap goes through SBUF->SBUF DMA.
                raw = ev1.tile([P, W], DT, tag="raw")
                nc.scalar.activation(raw[:], pt[:], ActFn.Copy)
                sw = ev1.tile([P, W], DT, tag="sw")
                nc.sync.dma_start(sw[0:64, :], raw[64:128, :])
                nc.sync.dma_start(sw[64:128, :], raw[0:64, :])
                t1 = ev1.tile([P, W], DT, tag="t1")
                nc.vector.tensor_tensor(
                    t1[:], sw[:], sin_sb[:, rsl], mybir.AluOpType.mult)
                rot = ev1.tile([P, W], DT, tag="rot")
                nc.vector.tensor_tensor(
                    rot[:], raw[:], cos_sb[:, rsl], mybir.AluOpType.mult)
                nc.vector.tensor_tensor(
                    rot[:], rot[:], t1[:], mybir.AluOpType.add)
                nc.sync.dma_start(dst[:, h, rsl], rot[:])

            for w in range(NW):
                xa = xp.tile([P, KO2, W], DT, tag="xa")
                xb = xp.tile([P, KO2, W], DT, tag="xb")
                nc.gpsimd.dma_start(xa[:], xT[:, w, 0:KO2])
                nc.gpsimd.dma_start(xb[:], xT[:, w, KO2:KO])
                rsl = slice(w * W, (w + 1) * W)
                if w == 0:
                    nc.gpsimd.dma_start(cos_sb[:], cosR[:])
                    nc.gpsimd.dma_start(sin_sb[:], sinS[:])
                for h in range(HL):
                    qk_proj(wq_sb, qT_d, h, xa, xb, rsl)
                if w == 0:
                    for c in range(KO // KCH):
                        ksl = slice(c * KCH, (c + 1) * KCH)
                        nc.gpsimd.dma_start(wk_sb[:, ksl], wkT[:, ksl])
                for h in range(HL):
                    qk_proj(wk_sb, kT_d, h, xa, xb, rsl)
                if w == 0:
                    for c in range(KO // KCH):
                        ksl = slice(c * KCH, (c + 1) * KCH)
                        nc.gpsimd.dma_start(wv_sb[:, ksl], wvT[:, ksl])
                for rs_ in range(W // P):
                    pt = ps1.tile([P, HL * HD], F32, tag="pv")
                    for k in range(KO):
                        xk_ = xa[:, k] if k < KO2 else xb[:, k - KO2]
                        nc.tensor.matmul(
                            pt[:], xk_[:, rs_ * P:(rs_ + 1) * P], wv_sb[:, k],
                            start=(k == 0), stop=(k == KO - 1),
                        )
                    vv = ev1.tile([P, HL * HD], DT, tag="vv")
                    nc.scalar.activation(vv[:], pt[:], ActFn.Copy)
                    nc.sync.dma_start(v_d[:, w * (W // P) + rs_, :], vv[:])

        # -------- phase 2+3: attention, split AllToAll, out-projection ------
        with (
            tc.tile_pool(name="tab2", bufs=1) as tab2,
            tc.tile_pool(name="att", bufs=2) as att,
            tc.tile_pool(name="up", bufs=4) as up,
            tc.tile_pool(name="ap3", bufs=2) as ap3,
            tc.tile_pool(name="wop", bufs=2) as wop,
            tc.tile_pool(name="yp", bufs=3) as yp,
            tc.tile_pool(name="ps23", bufs=2, space="PSUM") as ps23,
        ):
            ones_sb = tab2.tile([P, P], DT, tag="ones")
            nc.vector.memset(ones_sb[:], 1.0)
            mask_sb = tab2.tile([P, MOFF + QT], DT, tag="mask")
            nc.sync.dma_start(mask_sb[:], maskb[:])

            for b in range(B):
                vb = att.tile([P, NKT, HL * HD], DT, tag="vb")
                nc.gpsimd.dma_start(vb[:], v_d[:, b * NKT:(b + 1) * NKT, :])
                for h in range(HL):
                    kTb = att.tile([P, T], DT, tag="kTb")
                    nc.gpsimd.dma_start(kTb[:], kT_d[:, h, b * T:(b + 1) * T])
                    for qt in range(T // QT):
                        qTt = att.tile([P, QT], DT, tag="qTt")
                        nc.sync.dma_start(
                            qTt[:],
                            qT_d[:, h, b * T + qt * QT: b * T + (qt + 1) * QT])
                        po = ps23.tile([P, QT], F32, tag="po")
                        pd = ps23.tile([P, QT], F32, tag="pd")
                        nkt = (qt + 1) * (QT // P)

                        def consume(kt, pS, lo):
                            # exp -> (mask) -> accumulate o.T and denominator.
                            # Diagonal blocks only touch columns [lo:QT); the
                            # causal triangle sits in the first P of those.
                            u = up.tile([P, QT], DT, tag="u")
                            nc.scalar.activation(u[:, lo:], pS[:, lo:],
                                                 ActFn.Exp, scale=SCALE)
                            if lo or kt == qt * (QT // P):  # diagonal block
                                nc.vector.tensor_tensor(
                                    u[:, lo:lo + P], u[:, lo:lo + P],
                                    mask_sb[:, MOFF:MOFF + P],
                                    mybir.AluOpType.mult)
                            first, last = (kt == 0), (kt == nkt - 1)
                            nc.tensor.matmul(
                                po[:, lo:], vb[:, kt, h * HD:(h + 1) * HD],
                                u[:, lo:], start=first, stop=last)
                            nc.tensor.matmul(
                                pd[:, lo:], ones_sb[:], u[:, lo:],
                                start=first, stop=last)

                        # kt loop pipelined 2 deep: scores run 2 tiles ahead
                        # of exp/accumulate so the PE never waits on the ACT.
                        pending = []
                        for kt in range(nkt):
                            lo = max((kt - qt * (QT // P)) * P, 0)
                            pS = ps23.tile([P, QT], F32, tag="pS", bufs=4)
                            nc.tensor.matmul(
                                pS[:, lo:], kTb[:, kt * P:(kt + 1) * P],
                                qTt[:, lo:], start=True, stop=True,
                            )
                            pending.append((kt, pS, lo))
                            if len(pending) > 2:
                                consume(*pending.pop(0))
                        for it in pending:
                            consume(*it)

                        rec = up.tile([P, QT], F32, tag="rec")
                        nc.vector.reciprocal_approx_fast(rec[:], pd[:])
                        ot = up.tile([P, QT], DT, tag="ot")
                        nc.vector.tensor_tensor(
                            ot[:], po[:], rec[:], mybir.AluOpType.mult)
                        for j in range(2):
                            nc.sync.dma_start(
                                a2a_i[b][2 * qt + j, h * HD:(h + 1) * HD, :],
                                ot[:, j * RS2:(j + 1) * RS2])
                if b == 0:
                    # prefetch the first out-proj weight block during b=1
                    wot0 = wop.tile([P, KO, QT], DT, tag="wot")
                    nc.scalar.dma_start(wot0[:], woT[:, 0])

            # ---- collectives + output projection, overlapped per batch ----
            # aT loads stay on the gpsimd queue: its FIFO (a2a0, aT0, a2a1,
            # aT1) starts each load right after its collective, and the b=1
            # load's wait on the second collective blocks nothing behind it —
            # unlike the scalar ring, which phase-3 b=0 needs for weights.
            aT = []
            for b in range(B):
                if use_a2a:
                    nc.gpsimd.collective_compute(
                        "AllToAll",
                        mybir.AluOpType.bypass,
                        replica_groups=[list(range(NCORES))],
                        ins=[a2a_i[b][:]],
                        outs=[a2a_o[b][:]],
                    )
                    src_ab = a2a_o[b]
                else:
                    src_ab = a2a_i[b]
                aTb = ap3.tile([P, KO, RS2], DT, tag="aT", name=f"aT{b}")
                nc.gpsimd.dma_start(
                    aTb[:], src_ab[:].rearrange("s (i d) r -> d (s i) r", d=P))
                aT.append(aTb)

            for b in range(B):
                if b == 0:
                    cur = wot0  # prefetched during b=1 attention
                else:
                    cur = wop.tile([P, KO, QT], DT, tag="wot", name="wotc")
                    nc.scalar.dma_start(cur[:], woT[:, 0])
                for cb in range(NCB):
                    if cb + 1 < NCB:
                        nxt = wop.tile([P, KO, QT], DT, tag="wot", name="wotc")
                        nc.scalar.dma_start(nxt[:], woT[:, cb + 1])
                    for rs_ in range(RS2 // P):
                        pt = ps23.tile([P, QT], F32, tag="pS", bufs=4)
                        for k in range(KO):
                            nc.tensor.matmul(
                                pt[:], aT[b][:, k, rs_ * P:(rs_ + 1) * P],
                                cur[:, k],
                                start=(k == 0), stop=(k == KO - 1),
                            )
                        yt = yp.tile([P, QT], F32, tag="yt")
                        nc.scalar.activation(yt[:], pt[:], ActFn.Copy)
                        nc.sync.dma_start(
                            y[b * RS2 + rs_ * P: b * RS2 + (rs_ + 1) * P,
                              cb * QT:(cb + 1) * QT], yt[:])
                    if cb + 1 < NCB:
                        cur = nxt

    nc.compile()
    return nc


def _as_lhsT_tiles(w):
    """[M, K] row-major -> [P, K//P, M]: out[p, ko, m] = w[m, ko*P + p]."""
    M, K = w.shape
    return np.ascontiguousarray(
        w.reshape(M, K // P, P).transpose(2, 1, 0)).astype(BF16)


def prep_inputs(x, wq, wk, wv, wo, cfg=FULL):
    B, T, C, H, HD, HL, R, RS, KO, W, QT = _dims(cfg)
    NW = R // W
    NCB = C // QT
    MOFF = QT - P
    rope_perm = np.concatenate([np.arange(0, HD, 2), np.arange(1, HD, 2)])

    xflat = np.ascontiguousarray(x.reshape(R, C))
    # xT[p, w, ko, j] = x[w*W + j, ko*P + p]
    xT = np.ascontiguousarray(
        xflat.reshape(NW, W, KO, P).transpose(3, 0, 2, 1)).astype(BF16)
    # woT[p, cb, ko, j] = wo[cb*QT + j, ko*P + p]
    woT = np.ascontiguousarray(
        wo.reshape(NCB, QT, KO, P).transpose(3, 0, 2, 1)).astype(BF16)

    t = (np.arange(R) % T).astype(np.float64)
    cosR = np.broadcast_to(np.cos(t), (P, R)).astype(BF16)
    sin_row = np.sin(t)
    sinS = np.empty((P, R), np.float64)
    sinS[0:64, :] = -sin_row
    sinS[64:128, :] = sin_row
    sinS = sinS.astype(BF16)

    # mask[p, u] = 1 iff u >= p + MOFF; diagonal block with key-offset `off`
    # uses slice [MOFF-off : MOFF-off+QT] giving allowed = (qf >= kp + off)
    uu = np.arange(MOFF + QT)
    maskb = (uu[None, :] >= (np.arange(P)[:, None] + MOFF)).astype(BF16)

    per_core = []
    for m in range(NCORES):
        sl = slice(m * HL * HD, (m + 1) * HL * HD)
        wq_m = wq[sl].reshape(HL, HD, C)[:, rope_perm, :].reshape(HL * HD, C)
        wk_m = wk[sl].reshape(HL, HD, C)[:, rope_perm, :].reshape(HL * HD, C)
        per_core.append(dict(
            xT=xT,
            wqT=_as_lhsT_tiles(wq_m),
            wkT=_as_lhsT_tiles(wk_m),
            wvT=_as_lhsT_tiles(wv[sl]),
            woT=woT,
            cosR=cosR,
            sinS=sinS,
            maskb=maskb,
        ))
    return per_core


_NC_CACHE = None


def kernel(x, wq, wk, wv, wo):
    global _NC_CACHE
    cfg = FULL
    B, T, C = cfg["B"], cfg["T"], cfg["C"]
    RS2 = (B * T) // NCORES // B
    if _NC_CACHE is None:
        _NC_CACHE = build_nc(cfg)
    nc = _NC_CACHE
    in_maps = prep_inputs(
        np.asarray(x, np.float32), np.asarray(wq, np.float32),
        np.asarray(wk, np.float32), np.asarray(wv, np.float32),
        np.asarray(wo, np.float32), cfg)
    res = run_bass_kernel_spmd(nc, in_maps, core_ids=list(range(NCORES)))
    out = np.empty((B, T, C), np.float32)
    for m in range(NCORES):
        ym = res.results[m]["y"]
        for b in range(B):
            out[b, m * RS2:(m + 1) * RS2] = ym[b * RS2:(b + 1) * RS2]
    return out


# revision 4
# speedup vs baseline: 1.0014x; 1.0014x over previous
"""v7: v6 + fused a2a writes + yt eviction on DVE.  was v6: v2 + partial-width diagonal score blocks. Multi-head causal attention (B=2, T=2048, C=4096, H=32) on 8 Trainium2
NeuronCores, tensor-parallel over heads (Megatron-style). v2.

Per core m (4 heads each):
  phase 1: q/k/v projections from full x (weights column-sharded, host
           pre-transposed to lhsT layout, loaded in k-chunks so the first
           matmul starts ~10us in). RoPE at PSUM eviction (all rotary freqs
           == 1.0 here, so cos/sin are per-position scalars; head_dim is
           host-permuted to [evens, odds] so rotation pairs sit in partition
           halves; the half-swap runs through SBUF->SBUF DMA).
  phase 2: attention per (head, batch) with scores TRANSPOSED [k, q]:
           u = exp(scale * sT) (no max subtraction needed at these scales),
           causal-masked; o.T = v.T @ probs.T accumulates in PSUM; softmax
           denominator via an all-ones stationary matmul. The kt loop is
           software-pipelined 2 deep so the PE never waits on the exp.
  phase 3: two AllToAlls (one per batch element) redistribute o.T so each
           core owns ALL heads for a 256-row slice of each batch; the b=0
           collective and its output projection overlap with b=1 attention.
           y rows [0:256) = batch 0 slice, [256:512) = batch 1 slice.
Host gathers the 16 row-slices. Host does layout prep (transpose/cast) and
the final scatter only.
"""

import os
import sys

import numpy as np

for _p in ("/opt/trn_rl_repo", "/root/.axon_site/_ro/trn_rl_repo"):
    if os.path.isdir(_p) and _p not in sys.path:
        sys.path.insert(0, _p)

import ml_dtypes

import concourse.bacc as bacc
import concourse.bass as bass
import concourse.mybir as mybir
import concourse.tile as tile
from concourse.bass_utils import run_bass_kernel_spmd

BF16 = ml_dtypes.bfloat16
P = 128
NCORES = 8
DT = mybir.dt.bfloat16
F32 = mybir.dt.float32
ActFn = mybir.ActivationFunctionType

FULL = dict(B=2, T=2048, C=4096, H=32, W=512, QT=512)


def _dims(cfg):
    B, T, C, H = cfg["B"], cfg["T"], cfg["C"], cfg["H"]
    W, QT = cfg["W"], cfg["QT"]
    HD = C // H
    assert HD == P
    HL = H // NCORES
    R = B * T
    RS = R // NCORES
    KO = C // P
    return B, T, C, H, HD, HL, R, RS, KO, W, QT


def build_nc(cfg=FULL, repeat=1, use_a2a=True):
    B, T, C, H, HD, HL, R, RS, KO, W, QT = _dims(cfg)
    NW = R // W
    NKT = T // P
    NCB = C // QT
    SCALE = float(HD) ** -0.5
    MOFF = QT - P
    KO2 = KO // 2
    KCH = KO // 4           # weight-load chunk
    RS2 = RS // B           # rows per (core, batch)

    nc = bacc.Bacc(None, num_devices=NCORES)

    xT = nc.dram_tensor("xT", [P, NW, KO, W], DT, kind="ExternalInput")
    wqT = nc.dram_tensor("wqT", [P, KO, HL * HD], DT, kind="ExternalInput")
    wkT = nc.dram_tensor("wkT", [P, KO, HL * HD], DT, kind="ExternalInput")
    wvT = nc.dram_tensor("wvT", [P, KO, HL * HD], DT, kind="ExternalInput")
    woT = nc.dram_tensor("woT", [P, NCB, KO, QT], DT, kind="ExternalInput")
    cosR = nc.dram_tensor("cosR", [P, R], DT, kind="ExternalInput")
    sinS = nc.dram_tensor("sinS", [P, R], DT, kind="ExternalInput")
    maskb = nc.dram_tensor("maskb", [P, MOFF + QT], DT, kind="ExternalInput")
    y = nc.dram_tensor("y", [RS, C], F32, kind="ExternalOutput")

    qT_d = nc.dram_tensor("qT_d", [P, HL, R], DT)
    kT_d = nc.dram_tensor("kT_d", [P, HL, R], DT)
    v_d = nc.dram_tensor("v_d", [P, R // P, HL * HD], DT)
    a2a_i = [nc.dram_tensor(f"a2a_i{b}", [NCORES, HL * HD, RS2], DT)
             for b in range(B)]
    a2a_o = [nc.dram_tensor(f"a2a_o{b}", [NCORES, HL * HD, RS2], DT)
             for b in range(B)]

    with tile.TileContext(nc) as tc:
      for _rep in range(repeat):
        # ---------------- phase 1: q/k/v projections + rope ----------------
        with (
            tc.tile_pool(name="wp", bufs=1) as wp,
            tc.tile_pool(name="tab1", bufs=1) as tab1,
            tc.tile_pool(name="xp", bufs=2) as xp,
            tc.tile_pool(name="ev1", bufs=3) as ev1,
            tc.tile_pool(name="ps1", bufs=2, space="PSUM") as ps1,
        ):
            wq_sb = wp.tile([P, KO, HL * HD], DT, tag="wq")
            wk_sb = wp.tile([P, KO, HL * HD], DT, tag="wk")
            wv_sb = wp.tile([P, KO, HL * HD], DT, tag="wv")
            cos_sb = tab1.tile([P, R], DT, tag="cos")
            sin_sb = tab1.tile([P, R], DT, tag="sin")
            for c in range(KO // KCH):
                ksl = slice(c * KCH, (c + 1) * KCH)
                nc.gpsimd.dma_start(wq_sb[:, ksl], wqT[:, ksl])

            def qk_proj(wsb, dst, h, xa, xb, rsl):
                pt = ps1.tile([P, W], F32, tag="pqk")
                for k in range(KO):
                    xk_ = xa[:, k] if k < KO2 else xb[:, k - KO2]
                    nc.tensor.matmul(
                        pt[:], wsb[:, k, h * HD:(h + 1) * HD], xk_,
                        start=(k == 0), stop=(k == KO - 1),
                    )
                # rope: rot = raw*cos + swap(raw)*sinS (sign-split sin); the
                # partition half-swap goes through SBUF->SBUF DMA.
                raw = ev1.tile([P, W], DT, tag="raw")
                nc.scalar.activation(raw[:], pt[:], ActFn.Copy)
                sw = ev1.tile([P, W], DT, tag="sw")
                nc.sync.dma_start(sw[0:64, :], raw[64:128, :])
                nc.sync.dma_start(sw[64:128, :], raw[0:64, :])
                t1 = ev1.tile([P, W], DT, tag="t1")
                nc.vector.tensor_tensor(
                    t1[:], sw[:], sin_sb[:, rsl], mybir.AluOpType.mult)
                rot = ev1.tile([P, W], DT, tag="rot")
                nc.vector.tensor_tensor(
                    rot[:], raw[:], cos_sb[:, rsl], mybir.AluOpType.mult)
                nc.vector.tensor_tensor(
                    rot[:], rot[:], t1[:], mybir.AluOpType.add)
                nc.sync.dma_start(dst[:, h, rsl], rot[:])

            for w in range(NW):
                xa = xp.tile([P, KO2, W], DT, tag="xa")
                xb = xp.tile([P, KO2, W], DT, tag="xb")
                nc.gpsimd.dma_start(xa[:], xT[:, w, 0:KO2])
                nc.gpsimd.dma_start(xb[:], xT[:, w, KO2:KO])
                rsl = slice(w * W, (w + 1) * W)
                if w == 0:
                    nc.gpsimd.dma_start(cos_sb[:], cosR[:])
                    nc.gpsimd.dma_start(sin_sb[:], sinS[:])
                for h in range(HL):
                    qk_proj(wq_sb, qT_d, h, xa, xb, rsl)
                if w == 0:
                    for c in range(KO // KCH):
                        ksl = slice(c * KCH, (c + 1) * KCH)
                        nc.gpsimd.dma_start(wk_sb[:, ksl], wkT[:, ksl])
                for h in range(HL):
                    qk_proj(wk_sb, kT_d, h, xa, xb, rsl)
                if w == 0:
                    for c in range(KO // KCH):
                        ksl = slice(c * KCH, (c + 1) * KCH)
                        nc.gpsimd.dma_start(wv_sb[:, ksl], wvT[:, ksl])
                for rs_ in range(W // P):
                    pt = ps1.tile([P, HL * HD], F32, tag="pv")
                    for k in range(KO):
                        xk_ = xa[:, k] if k < KO2 else xb[:, k - KO2]
                        nc.tensor.matmul(
                            pt[:], xk_[:, rs_ * P:(rs_ + 1) * P], wv_sb[:, k],
                            start=(k == 0), stop=(k == KO - 1),
                        )
                    vv = ev1.tile([P, HL * HD], DT, tag="vv")
                    nc.scalar.activation(vv[:], pt[:], ActFn.Copy)
                    nc.sync.dma_start(v_d[:, w * (W // P) + rs_, :], vv[:])

        # -------- phase 2+3: attention, split AllToAll, out-projection ------
        with (
            tc.tile_pool(name="tab2", bufs=1) as tab2,
            tc.tile_pool(name="att", bufs=2) as att,
            tc.tile_pool(name="up", bufs=4) as up,
            tc.tile_pool(name="ap3", bufs=2) as ap3,
            tc.tile_pool(name="wop", bufs=2) as wop,
            tc.tile_pool(name="yp", bufs=3) as yp,
            tc.tile_pool(name="ps23", bufs=2, space="PSUM") as ps23,
        ):
            ones_sb = tab2.tile([P, P], DT, tag="ones")
            nc.vector.memset(ones_sb[:], 1.0)
            mask_sb = tab2.tile([P, MOFF + QT], DT, tag="mask")
            nc.sync.dma_start(mask_sb[:], maskb[:])

            for b in range(B):
                vb = att.tile([P, NKT, HL * HD], DT, tag="vb")
                nc.gpsimd.dma_start(vb[:], v_d[:, b * NKT:(b + 1) * NKT, :])
                for h in range(HL):
                    kTb = att.tile([P, T], DT, tag="kTb")
                    nc.gpsimd.dma_start(kTb[:], kT_d[:, h, b * T:(b + 1) * T])
                    for qt in range(T // QT):
                        qTt = att.tile([P, QT], DT, tag="qTt")
                        nc.sync.dma_start(
                            qTt[:],
                            qT_d[:, h, b * T + qt * QT: b * T + (qt + 1) * QT])
                        po = ps23.tile([P, QT], F32, tag="po")
                        pd = ps23.tile([P, QT], F32, tag="pd")
                        nkt = (qt + 1) * (QT // P)

                        def consume(kt, pS, lo):
                            # exp -> (mask) -> accumulate o.T and denominator.
                            # Diagonal blocks only touch columns [lo:QT); the
                            # causal triangle sits in the first P of those.
                            u = up.tile([P, QT], DT, tag="u")
                            nc.scalar.activation(u[:, lo:], pS[:, lo:],
                                                 ActFn.Exp, scale=SCALE)
                            if lo or kt == qt * (QT // P):  # diagonal block
                                nc.vector.tensor_tensor(
                                    u[:, lo:lo + P], u[:, lo:lo + P],
                                    mask_sb[:, MOFF:MOFF + P],
                                    mybir.AluOpType.mult)
                            first, last = (kt == 0), (kt == nkt - 1)
                            nc.tensor.matmul(
                                po[:, lo:], vb[:, kt, h * HD:(h + 1) * HD],
                                u[:, lo:], start=first, stop=last)
                            nc.tensor.matmul(
                                pd[:, lo:], ones_sb[:], u[:, lo:],
                                start=first, stop=last)

                        # kt loop pipelined 2 deep: scores run 2 tiles ahead
                        # of exp/accumulate so the PE never waits on the ACT.
                        pending = []
                        for kt in range(nkt):
                            lo = max((kt - qt * (QT // P)) * P, 0)
                            pS = ps23.tile([P, QT], F32, tag="pS", bufs=4)
                            nc.tensor.matmul(
                                pS[:, lo:], kTb[:, kt * P:(kt + 1) * P],
                                qTt[:, lo:], start=True, stop=True,
                            )
                            pending.append((kt, pS, lo))
                            if len(pending) > 2:
                                consume(*pending.pop(0))
                        for it in pending:
                            consume(*it)

                        rec = up.tile([P, QT], F32, tag="rec")
                        nc.vector.reciprocal_approx_fast(rec[:], pd[:])
                        ot = up.tile([P, QT], DT, tag="ot")
                        nc.vector.tensor_tensor(
                            ot[:], po[:], rec[:], mybir.AluOpType.mult)
                        # one DMA covers both destination cores: the DRAM
                        # AP is rearranged so its flattened (core, col) order
                        # matches ot's column order.
                        nc.sync.dma_start(
                            a2a_i[b][2 * qt:2 * qt + 2,
                                     h * HD:(h + 1) * HD, :].rearrange(
                                         "j d r -> d j r"),
                            ot[:].rearrange("p (j r) -> p j r", j=2))
                if b == 0:
                    # prefetch the first out-proj weight block during b=1
                    wot0 = wop.tile([P, KO, QT], DT, tag="wot")
                    nc.scalar.dma_start(wot0[:], woT[:, 0])

            # ---- collectives + output projection, overlapped per batch ----
            # aT loads stay on the gpsimd queue: its FIFO (a2a0, aT0, a2a1,
            # aT1) starts each load right after its collective, and the b=1
            # load's wait on the second collective blocks nothing behind it —
            # unlike the scalar ring, which phase-3 b=0 needs for weights.
            aT = []
            for b in range(B):
                if use_a2a:
                    nc.gpsimd.collective_compute(
                        "AllToAll",
                        mybir.AluOpType.bypass,
                        replica_groups=[list(range(NCORES))],
                        ins=[a2a_i[b][:]],
                        outs=[a2a_o[b][:]],
                    )
                    src_ab = a2a_o[b]
                else:
                    src_ab = a2a_i[b]
                aTb = ap3.tile([P, KO, RS2], DT, tag="aT", name=f"aT{b}")
                nc.gpsimd.dma_start(
                    aTb[:], src_ab[:].rearrange("s (i d) r -> d (s i) r", d=P))
                aT.append(aTb)

            for b in range(B):
                if b == 0:
                    cur = wot0  # prefetched during b=1 attention
                else:
                    cur = wop.tile([P, KO, QT], DT, tag="wot", name="wotc")
                    nc.scalar.dma_start(cur[:], woT[:, 0])
                for cb in range(NCB):
                    if cb + 1 < NCB:
                        nxt = wop.tile([P, KO, QT], DT, tag="wot", name="wotc")
                        nc.scalar.dma_start(nxt[:], woT[:, cb + 1])
                    for rs_ in range(RS2 // P):
                        pt = ps23.tile([P, QT], F32, tag="pS", bufs=4)
                        for k in range(KO):
                            nc.tensor.matmul(
                                pt[:], aT[b][:, k, rs_ * P:(rs_ + 1) * P],
                                cur[:, k],
                                start=(k == 0), stop=(k == KO - 1),
                            )
                        yt = yp.tile([P, QT], F32, tag="yt")
                        nc.vector.tensor_copy(yt[:], pt[:])
                        nc.sync.dma_start(
                            y[b * RS2 + rs_ * P: b * RS2 + (rs_ + 1) * P,
                              cb * QT:(cb + 1) * QT], yt[:])
                    if cb + 1 < NCB:
                        cur = nxt

    nc.compile()
    return nc


def _as_lhsT_tiles(w):
    """[M, K] row-major -> [P, K//P, M]: out[p, ko, m] = w[m, ko*P + p]."""
    M, K = w.shape
    return np.ascontiguousarray(
        w.reshape(M, K // P, P).transpose(2, 1, 0)).astype(BF16)


def prep_inputs(x, wq, wk, wv, wo, cfg=FULL):
    B, T, C, H, HD, HL, R, RS, KO, W, QT = _dims(cfg)
    NW = R // W
    NCB = C // QT
    MOFF = QT - P
    rope_perm = np.concatenate([np.arange(0, HD, 2), np.arange(1, HD, 2)])

    xflat = np.ascontiguousarray(x.reshape(R, C))
    # xT[p, w, ko, j] = x[w*W + j, ko*P + p]
    xT = np.ascontiguousarray(
        xflat.reshape(NW, W, KO, P).transpose(3, 0, 2, 1)).astype(BF16)
    # woT[p, cb, ko, j] = wo[cb*QT + j, ko*P + p]
    woT = np.ascontiguousarray(
        wo.reshape(NCB, QT, KO, P).transpose(3, 0, 2, 1)).astype(BF16)

    t = (np.arange(R) % T).astype(np.float64)
    cosR = np.broadcast_to(np.cos(t), (P, R)).astype(BF16)
    sin_row = np.sin(t)
    sinS = np.empty((P, R), np.float64)
    sinS[0:64, :] = -sin_row
    sinS[64:128, :] = sin_row
    sinS = sinS.astype(BF16)

    # mask[p, u] = 1 iff u >= p + MOFF; diagonal block with key-offset `off`
    # uses slice [MOFF-off : MOFF-off+QT] giving allowed = (qf >= kp + off)
    uu = np.arange(MOFF + QT)
    maskb = (uu[None, :] >= (np.arange(P)[:, None] + MOFF)).astype(BF16)

    per_core = []
    for m in range(NCORES):
        sl = slice(m * HL * HD, (m + 1) * HL * HD)
        wq_m = wq[sl].reshape(HL, HD, C)[:, rope_perm, :].reshape(HL * HD, C)
        wk_m = wk[sl].reshape(HL, HD, C)[:, rope_perm, :].reshape(HL * HD, C)
        per_core.append(dict(
            xT=xT,
            wqT=_as_lhsT_tiles(wq_m),
            wkT=_as_lhsT_tiles(wk_m),
            wvT=_as_lhsT_tiles(wv[sl]),
            woT=woT,
            cosR=cosR,
            sinS=sinS,
            maskb=maskb,
        ))
    return per_core


_NC_CACHE = None


def kernel(x, wq, wk, wv, wo):
    global _NC_CACHE
    cfg = FULL
    B, T, C = cfg["B"], cfg["T"], cfg["C"]
    RS2 = (B * T) // NCORES // B
    if _NC_CACHE is None:
        _NC_CACHE = build_nc(cfg)
    nc = _NC_CACHE
    in_maps = prep_inputs(
        np.asarray(x, np.float32), np.asarray(wq, np.float32),
        np.asarray(wk, np.float32), np.asarray(wv, np.float32),
        np.asarray(wo, np.float32), cfg)
    res = run_bass_kernel_spmd(nc, in_maps, core_ids=list(range(NCORES)))
    out = np.empty((B, T, C), np.float32)
    for m in range(NCORES):
        ym = res.results[m]["y"]
        for b in range(B):
            out[b, m * RS2:(m + 1) * RS2] = ym[b * RS2:(b + 1) * RS2]
    return out
